# revision 17
# baseline (speedup 1.0000x reference)
"""DiffAttention Trainium2 kernel (8-core SPMD), v3.

Problem shapes: b=4, t=1024, d=1024, H=16 v-heads (2H=32 q/k heads), E=64.
Sharding: batch x head-block. Core c handles batch c//2 and v-heads
[8*(c%2), 8*(c%2)+8)  (= q/k heads [16*(c%2), 16*(c%2)+16)).

Design (per core):
  - fp16 matmul pipeline for projections and scores; bf16 for the
    post-exp path (e, V panel, O) because scores reach ~26 so exp(s)
    overflows fp16's range.
  - Scores per (pair, qblock, kchunk): two row-packed concurrent 64-row
    matmuls (pos rows 0-63, neg rows 64-127) -> s[128,2,512] fp32 PSUM;
    one ACTIVATE(exp) FD=1024 -> e bf16.
  - V panel per k-chunk is [1 | V | -1/lam] (66 cols), shared stationary
    for both AV matmuls: o_pos=[den_pos; O_pos; junk], o_neg=[junk';
    O_neg; -den_neg/lam].  After transposing O^T, the combine is just
    two batched reciprocals + TS/STT per 128-row block: the -1/lam
    column makes rn = -lam/den_neg directly.
  - AV runs one e-tile behind (6-deep e pool) smoothly across q-block
    and pair boundaries; per-qblock O transposes go through the DMA
    transpose crossbar (issued from the otherwise-idle GpSimd queue),
    keeping the PE free; the last pair uses PE transposes (PE is
    underloaded there and it shortens the tail).
  - QK projections of pair p+1 interleave into pair p's exp bubbles;
    V projection interleaves into pair 0 / q-block 0.
  - Input DMA issue order = dependency priority, split across engine
    queues: sync carries wq[pair0]+xq, gpsimd wk[pair0]+xk then the
    remaining w columns, vector xv+wv.

PSUM: s 2 banks x2 + o_pos/o_neg 1 bank x2 + proj accumulators 1x2 = 8.
"""

import numpy as np
from contextlib import ExitStack

import concourse.bass as bass
import concourse.tile as tile
from concourse import bacc, mybir
from concourse.bass_utils import run_bass_kernel_spmd
from concourse.masks import make_identity

F32 = mybir.dt.float32
F16 = mybir.dt.float16
BF16 = mybir.dt.bfloat16
EXP = mybir.ActivationFunctionType.Exp

E = 64          # per-head embed
H = 16          # global v-heads
B = 4           # batch
T = 1024        # sequence length
D = 1024        # model dim
N_CORES = 8

# per-core sizes
NQKH = 16                  # local q/k heads
PAIRS = NQKH // 2          # local head pairs / v heads
HE = NQKH * E              # 1024, q/k projection width
VHE = PAIRS * E            # 512, v projection width / output width
DC = D // 128              # contraction chunks
KC = T // 128              # key-position chunks
QB = T // 512              # query blocks of 512
QT4 = 4                    # 128-q-tiles per q block
EW = E + 2                 # live V panel width: [1 | V | -1/lam]
EWP = 80                   # padded to 16-multiple for DMA transpose


def build_bass(mm_dt=F16):
    nc = bacc.Bacc("TRN2", target_bir_lowering=False, debug=False,
                   num_devices=N_CORES)

    xqT = nc.dram_tensor("xqT", [D, T], mm_dt, kind="ExternalInput").ap()
    xkT = nc.dram_tensor("xkT", [D, T], mm_dt, kind="ExternalInput").ap()
    xvT = nc.dram_tensor("xvT", [D, T], mm_dt, kind="ExternalInput").ap()
    wqT = nc.dram_tensor("wqT", [D, HE], mm_dt, kind="ExternalInput").ap()
    wkT = nc.dram_tensor("wkT", [D, HE], mm_dt, kind="ExternalInput").ap()
    wvT = nc.dram_tensor("wvT", [D, VHE], mm_dt, kind="ExternalInput").ap()
    nlam = nc.dram_tensor("nlam", [128, PAIRS], F32, kind="ExternalInput").ap()
    out = nc.dram_tensor("out", [T, VHE], F32, kind="ExternalOutput").ap()

    mm = nc.tensor.matmul

    with tile.TileContext(nc) as tc, ExitStack() as ctx:
        res = ctx.enter_context(tc.tile_pool(name="res", bufs=1))
        pin = ctx.enter_context(tc.tile_pool(name="pin", bufs=1))
        vpsum = ctx.enter_context(tc.tile_pool(name="vpsum", bufs=1,
                                               space="PSUM"))
        qpsum = ctx.enter_context(tc.tile_pool(name="qpsum", bufs=1,
                                               space="PSUM"))
        s_pool = ctx.enter_context(tc.tile_pool(name="s", bufs=2,
                                                space="PSUM"))
        o_pool = ctx.enter_context(tc.tile_pool(name="o", bufs=2,
                                                space="PSUM"))
        pexp_pool = ctx.enter_context(tc.tile_pool(name="pexp", bufs=12))
        post_pool = ctx.enter_context(tc.tile_pool(name="post", bufs=6))

        # resident input tiles (fp16)
        xq_sb = [[pin.tile([128, 512], mm_dt, tag=f"xq{i}_{h}",
                           name=f"xq{i}_{h}") for h in range(2)]
                 for i in range(DC)]
        wq_sb = [pin.tile([128, HE], mm_dt, tag=f"wq{i}", name=f"wq{i}")
                 for i in range(DC)]
        xk_sb = [[pin.tile([128, 512], mm_dt, tag=f"xk{i}_{h}",
                           name=f"xk{i}_{h}") for h in range(2)]
                 for i in range(DC)]
        wk_sb = [pin.tile([128, HE], mm_dt, tag=f"wk{i}", name=f"wk{i}")
                 for i in range(DC)]
        xv_sb = [[pin.tile([128, 512], mm_dt, tag=f"xv{i}_{h}",
                           name=f"xv{i}_{h}") for h in range(2)]
                 for i in range(DC)]
        wv_sb = [pin.tile([128, VHE], mm_dt, tag=f"wv{i}", name=f"wv{i}")
                 for i in range(DC)]

        # DMA issue plan.  Two issue queues (sync + gpsimd); within a
        # queue, issue order ~= transfer priority.  Critical path for the
        # first attention slot: wq0/wk0 + the tq0 halves of xq/xk.
        def rr(i):
            return slice(i * 128, (i + 1) * 128)
        for i in range(DC):
            nc.sync.dma_start(out=wq_sb[i][:, 0:128], in_=wqT[rr(i), 0:128])
        for h in range(2):
            for i in range(DC):
                nc.sync.dma_start(out=xq_sb[i][h],
                                  in_=xqT[rr(i), h * 512:(h + 1) * 512])
        for i in range(DC):
            nc.gpsimd.dma_start(out=wk_sb[i][:, 0:128], in_=wkT[rr(i), 0:128])
        for h in range(2):
            for i in range(DC):
                nc.gpsimd.dma_start(out=xk_sb[i][h],
                                    in_=xkT[rr(i), h * 512:(h + 1) * 512])
        for h in range(2):
            for i in range(DC):
                nc.gpsimd.dma_start(out=xv_sb[i][h],
                                    in_=xvT[rr(i), h * 512:(h + 1) * 512])
        for i in range(DC):
            nc.gpsimd.dma_start(out=wv_sb[i], in_=wvT[rr(i), :])
        for i in range(DC):
            nc.sync.dma_start(out=wq_sb[i][:, 128:HE], in_=wqT[rr(i), 128:HE])
            nc.sync.dma_start(out=wk_sb[i][:, 128:HE], in_=wkT[rr(i), 128:HE])

        # resident intermediates
        QT = [res.tile([128, T], mm_dt, tag=f"QT{i}", name=f"QT{i}")
              for i in range(PAIRS)]
        KT = [res.tile([128, T], mm_dt, tag=f"KT{i}", name=f"KT{i}")
              for i in range(PAIRS)]
        # V panel per k-chunk: col 0 = 1.0, cols 1..64 = V, col 65 = -1/lam
        VB = [res.tile([128, PAIRS, EWP], BF16, tag=f"VB{i}",
                       name=f"VB{i}") for i in range(KC)]
        ident = res.tile([128, 128], BF16, tag="ident", name="ident")
        make_identity(nc, ident)
        nlam_sb = res.tile([128, PAIRS], F32, tag="nlam", name="nlam_sb")
        nc.sync.dma_start(out=nlam_sb, in_=nlam)
        for i in range(KC):
            nc.vector.memset(VB[i][:, :, 0:1], 1.0)
            nc.vector.memset(VB[i][:, :, EW:EWP], 0.0)
            nc.vector.tensor_copy(VB[i][:, :, E + 1:E + 2], nlam_sb)

        # ---- V projection for one 128-key chunk ----
        def emit_v_chunk(tcn):
            ps = vpsum.tile([128, 512], F32, tag="ps", name="psv")
            h, t4 = tcn // 4, tcn % 4
            for dc in range(DC):
                mm(ps, xv_sb[dc][h][:, t4 * 128:(t4 + 1) * 128],
                   wv_sb[dc],
                   start=(dc == 0), stop=(dc == DC - 1))
            nc.vector.tensor_copy(VB[tcn][:, :, 1:E + 1],
                                  ps.rearrange("p (h e) -> p h e", e=E))

        # ---- QK projection group: one (pair, q|k, tq-half) = 8 dc-mms ----
        def proj_group(p, xw, tq):
            csl = slice(p * 128, (p + 1) * 128)
            x_sb, w_sb, OUT = ((xq_sb, wq_sb, QT) if xw == "q"
                               else (xk_sb, wk_sb, KT))
            ps = qpsum.tile([128, 512], F32, tag="ps", name="psqk")
            return [(ps, w_sb[dc], csl, x_sb, tq, dc, OUT[p])
                    for dc in range(DC)]

        def emit_proj(op):
            ps, w, csl, x, tq, dc, dst = op
            mm(ps, w[:, csl], x[dc][tq],
               start=(dc == 0), stop=(dc == DC - 1))
            if dc == DC - 1:
                nc.vector.tensor_copy(dst[:, tq * 512:(tq + 1) * 512], ps)

        # pair-0 tq0-half QK projection upfront (overlaps input DMA);
        # the tq1 halves run inside pair-0 qb0's slots.
        for op in proj_group(0, "q", 0) + proj_group(0, "k", 0):
            emit_proj(op)

        # ---- attention pipeline ----
        # One slot = (pair, qblock, kchunk): emit scores + exp, the AV
        # matmuls of the previous slot's e-tile, and filler projections.
        prev = None  # (e_tile, p, qb, kc)
        cur_o = {}   # (p, qb) -> (o_pos, o_neg)

        def emit_av_and_post(rec):
            e, pp_, qq_, kk_ = rec
            if kk_ == 0:
                o_pos = o_pool.tile([EWP, 512], F32, tag="o", name="o_pos")
                o_neg = o_pool.tile([EWP, 512], F32, tag="o", name="o_neg")
                cur_o[(pp_, qq_)] = (o_pos, o_neg)
            o_pos, o_neg = cur_o[(pp_, qq_)]
            first = (kk_ == 0)
            last = (kk_ == KC - 1)
            mm(o_pos, VB[kk_][:, pp_, :], e[:, 0, :], start=first, stop=last)
            mm(o_neg, VB[kk_][:, pp_, :], e[:, 1, :], start=first, stop=last)
            if last:
                emit_post(pp_, qq_, o_pos, o_neg)
                del cur_o[(pp_, qq_)]

        def emit_post(pp_, qq_, o_pos, o_neg):
            # o_pos rows: [den_pos, O_pos(64), junk]
            # o_neg rows: [junk,    O_neg(64), -den_neg/lam]
            osb = post_pool.tile([EWP, 2, 512], BF16, tag="osb", name="osb")
            nc.vector.tensor_copy(osb[:, 0, :], o_pos)
            nc.vector.tensor_copy(osb[:, 1, :], o_neg)
            tr = o_pool.tile([128, 2, QT4, EWP], BF16, tag="o", name="tr")
            for qt in range(QT4):
                tsl = slice(qt * 128, (qt + 1) * 128)
                nc.tensor.transpose(tr[:, 0, qt, :], osb[:, 0, tsl],
                                    ident[0:EWP, 0:EWP])
                nc.tensor.transpose(tr[:, 1, qt, :], osb[:, 1, tsl],
                                    ident[0:EWP, 0:EWP])
            rp = post_pool.tile([128, QT4], F32, tag="rp", name="rp")
            rn = post_pool.tile([128, QT4], F32, tag="rn", name="rn")
            nc.vector.reciprocal(rp, tr[:, 0, :, 0:1])
            nc.vector.reciprocal(rn, tr[:, 1, :, E + 1:E + 2])
            ot = post_pool.tile([128, QT4, E], F32, tag="ot", name="ot")
            for qt in range(QT4):
                nc.vector.tensor_scalar_mul(ot[:, qt, :],
                                            tr[:, 0, qt, 1:E + 1],
                                            rp[:, qt:qt + 1])
                nc.vector.scalar_tensor_tensor(
                    ot[:, qt, :], tr[:, 1, qt, 1:E + 1], rn[:, qt:qt + 1],
                    ot[:, qt, :],
                    op0=mybir.AluOpType.mult,
                    op1=mybir.AluOpType.add)
            nc.sync.dma_start(
                out=out[qq_ * 512:(qq_ + 1) * 512, pp_ * E:(pp_ + 1) * E]
                .rearrange("(qt r) e -> r qt e", qt=QT4),
                in_=ot)

        pending = []
        pi = 0
        for p in range(PAIRS):
            for qb in range(QB):
                qsl = slice(qb * 512, (qb + 1) * 512)
                # schedule projection groups so their CASTs land exactly
                # when first needed: pair p's tq1 halves during its own
                # qb0 (k-tq1 ready at kc4), pair p+1's tq0 halves during
                # pair p's qb1.
                if qb == 0:
                    pending += proj_group(p, "k", 1) + proj_group(p, "q", 1)
                elif p + 1 < PAIRS:
                    pending += (proj_group(p + 1, "q", 0)
                                + proj_group(p + 1, "k", 0))
                for kc in range(KC):
                    ksl = slice(kc * 128, (kc + 1) * 128)
                    s = s_pool.tile([128, 2, 512], F32, tag="s", name="s")
                    mm(s[:, 0, :], KT[p][0:64, ksl], QT[p][0:64, qsl],
                       start=True, stop=True, tile_position=(0, 0))
                    mm(s[:, 1, :], KT[p][64:128, ksl], QT[p][64:128, qsl],
                       start=True, stop=True, tile_position=(64, 0))
                    e = pexp_pool.tile([128, 2, 512], BF16, tag="e",
                                       name="e")
                    nc.scalar.activation(e, s, EXP)
                    if prev is not None:
                        emit_av_and_post(prev)
                    # fill the exp-wait bubble with V/projection work
                    if p == 0 and qb == 0:
                        emit_v_chunk(kc)
                    for _ in range(2):
                        if pi < len(pending):
                            emit_proj(pending[pi])
                            pi += 1
                    prev = (e, p, qb, kc)
            # safety drain (normally a no-op: groups balance exactly)
            while pi < len(pending):
                emit_proj(pending[pi])
                pi += 1
        # flush the last e-tile
        emit_av_and_post(prev)

    nc.compile()
    return nc


def make_in_maps(q_input, k_input, v_input, Wq, Wk, Wv, L):
    scale = np.float32(E ** -0.25)
    lam = (0.2 + np.exp(np.float32(L[0] @ L[1]))
           - np.exp(np.float32(L[2] @ L[3])))
    ninvlam = np.full((128, PAIRS), -1.0 / lam, np.float32)
    in_maps = []
    for c in range(N_CORES):
        b, hb = c // 2, c % 2
        in_maps.append({
            "xqT": np.ascontiguousarray(q_input[b].T).astype(np.float16),
            "xkT": np.ascontiguousarray(k_input[b].T).astype(np.float16),
            "xvT": np.ascontiguousarray(v_input[b].T).astype(np.float16),
            "wqT": (np.ascontiguousarray(Wq[1024 * hb:1024 * (hb + 1), :].T)
                    * scale).astype(np.float16),
            "wkT": (np.ascontiguousarray(Wk[1024 * hb:1024 * (hb + 1), :].T)
                    * scale).astype(np.float16),
            "wvT": np.ascontiguousarray(
                Wv[512 * hb:512 * (hb + 1), :].T).astype(np.float16),
            "nlam": ninvlam,
        })
    return in_maps


_NC_CACHE = {}


def get_nc(mm_dt=F16):
    key = str(mm_dt)
    if key not in _NC_CACHE:
        _NC_CACHE[key] = build_bass(mm_dt)
    return _NC_CACHE[key]


def kernel(q_input, k_input, v_input, Wq, Wk, Wv, L, _trace=False):
    q_input = np.asarray(q_input, np.float32)
    k_input = np.asarray(k_input, np.float32)
    v_input = np.asarray(v_input, np.float32)
    Wq = np.asarray(Wq, np.float32)
    Wk = np.asarray(Wk, np.float32)
    Wv = np.asarray(Wv, np.float32)
    L = np.asarray(L, np.float32)

    nc = get_nc()
    in_maps = make_in_maps(q_input, k_input, v_input, Wq, Wk, Wv, L)
    res = run_bass_kernel_spmd(nc, in_maps, list(range(N_CORES)), trace=_trace)

    full = np.empty((B, T, H * E), np.float32)
    for c in range(N_CORES):
        b, hb = c // 2, c % 2
        full[b, :, 512 * hb:512 * (hb + 1)] = res.results[c]["out"]
    if _trace:
        return full, res
    return full


# revision 18
# speedup vs baseline: 1.0731x; 1.0731x over previous
"""DiffAttention Trainium2 kernel (8-core SPMD), v3.

Problem shapes: b=4, t=1024, d=1024, H=16 v-heads (2H=32 q/k heads), E=64.
Sharding: batch x head-block. Core c handles batch c//2 and v-heads
[8*(c%2), 8*(c%2)+8)  (= q/k heads [16*(c%2), 16*(c%2)+16)).

Design (per core):
  - fp16 matmul pipeline for projections and scores; bf16 for the
    post-exp path (e, V panel, O) because scores reach ~26 so exp(s)
    overflows fp16's range.
  - Scores per (pair, qblock, kchunk): two row-packed concurrent 64-row
    matmuls (pos rows 0-63, neg rows 64-127) -> s[128,2,512] fp32 PSUM;
    one ACTIVATE(exp) FD=1024 -> e bf16.
  - V panel per k-chunk is [1 | V | -1/lam] (66 cols), shared stationary
    for both AV matmuls: o_pos=[den_pos; O_pos; junk], o_neg=[junk';
    O_neg; -den_neg/lam].  After transposing O^T, the combine is just
    two batched reciprocals + TS/STT per 128-row block: the -1/lam
    column makes rn = -lam/den_neg directly.
  - AV runs one e-tile behind (6-deep e pool) smoothly across q-block
    and pair boundaries; per-qblock O transposes go through the DMA
    transpose crossbar (issued from the otherwise-idle GpSimd queue),
    keeping the PE free; the last pair uses PE transposes (PE is
    underloaded there and it shortens the tail).
  - QK projections of pair p+1 interleave into pair p's exp bubbles;
    V projection interleaves into pair 0 / q-block 0.
  - Input DMA issue order = dependency priority, split across engine
    queues: sync carries wq[pair0]+xq, gpsimd wk[pair0]+xk then the
    remaining w columns, vector xv+wv.

PSUM: s 2 banks x2 + o_pos/o_neg 1 bank x2 + proj accumulators 1x2 = 8.
"""

import numpy as np
from contextlib import ExitStack

import concourse.bass as bass
import concourse.tile as tile
from concourse import bacc, mybir
from concourse.bass_utils import run_bass_kernel_spmd
from concourse.masks import make_identity

F32 = mybir.dt.float32
F16 = mybir.dt.float16
BF16 = mybir.dt.bfloat16
EXP = mybir.ActivationFunctionType.Exp

E = 64          # per-head embed
H = 16          # global v-heads
B = 4           # batch
T = 1024        # sequence length
D = 1024        # model dim
N_CORES = 8

# per-core sizes
NQKH = 16                  # local q/k heads
PAIRS = NQKH // 2          # local head pairs / v heads
HE = NQKH * E              # 1024, q/k projection width
VHE = PAIRS * E            # 512, v projection width / output width
DC = D // 128              # contraction chunks
KC = T // 128              # key-position chunks
QB = T // 512              # query blocks of 512
QT4 = 4                    # 128-q-tiles per q block
EW = E + 2                 # live V panel width: [1 | V | -1/lam]
EWP = 80                   # padded to 16-multiple for DMA transpose


def build_bass(mm_dt=F16):
    nc = bacc.Bacc("TRN2", target_bir_lowering=False, debug=False,
                   num_devices=N_CORES)

    xqT = nc.dram_tensor("xqT", [D, T], mm_dt, kind="ExternalInput").ap()
    xkT = nc.dram_tensor("xkT", [D, T], mm_dt, kind="ExternalInput").ap()
    xvT = nc.dram_tensor("xvT", [D, T], mm_dt, kind="ExternalInput").ap()
    wqT = nc.dram_tensor("wqT", [D, HE], mm_dt, kind="ExternalInput").ap()
    wkT = nc.dram_tensor("wkT", [D, HE], mm_dt, kind="ExternalInput").ap()
    wvT = nc.dram_tensor("wvT", [D, VHE], mm_dt, kind="ExternalInput").ap()
    nlam = nc.dram_tensor("nlam", [128, PAIRS], F32, kind="ExternalInput").ap()
    out = nc.dram_tensor("out", [T, VHE], F32, kind="ExternalOutput").ap()

    mm = nc.tensor.matmul

    with tile.TileContext(nc) as tc, ExitStack() as ctx:
        res = ctx.enter_context(tc.tile_pool(name="res", bufs=1))
        pin = ctx.enter_context(tc.tile_pool(name="pin", bufs=1))
        ppsum = ctx.enter_context(tc.tile_pool(name="ppsum", bufs=2,
                                               space="PSUM"))
        s_pool = ctx.enter_context(tc.tile_pool(name="s", bufs=2,
                                                space="PSUM"))
        o_pool = ctx.enter_context(tc.tile_pool(name="o", bufs=2,
                                                space="PSUM"))
        pexp_pool = ctx.enter_context(tc.tile_pool(name="pexp", bufs=12))
        post_pool = ctx.enter_context(tc.tile_pool(name="post", bufs=6))

        # resident input tiles (fp16)
        xq_sb = [[pin.tile([128, 512], mm_dt, tag=f"xq{i}_{h}",
                           name=f"xq{i}_{h}") for h in range(2)]
                 for i in range(DC)]
        wq_sb = [pin.tile([128, HE], mm_dt, tag=f"wq{i}", name=f"wq{i}")
                 for i in range(DC)]
        xk_sb = [[pin.tile([128, 512], mm_dt, tag=f"xk{i}_{h}",
                           name=f"xk{i}_{h}") for h in range(2)]
                 for i in range(DC)]
        wk_sb = [pin.tile([128, HE], mm_dt, tag=f"wk{i}", name=f"wk{i}")
                 for i in range(DC)]
        xv_sb = [[pin.tile([128, 512], mm_dt, tag=f"xv{i}_{h}",
                           name=f"xv{i}_{h}") for h in range(2)]
                 for i in range(DC)]
        wv_sb = [pin.tile([128, VHE], mm_dt, tag=f"wv{i}", name=f"wv{i}")
                 for i in range(DC)]

        # DMA issue plan.  Two issue queues (sync + gpsimd); within a
        # queue, issue order ~= transfer priority.  Critical path for the
        # first attention slot: wq0/wk0 + the tq0 halves of xq/xk.
        def rr(i):
            return slice(i * 128, (i + 1) * 128)
        for i in range(DC):
            nc.sync.dma_start(out=wq_sb[i][:, 0:128], in_=wqT[rr(i), 0:128])
        for h in range(2):
            for i in range(DC):
                nc.sync.dma_start(out=xq_sb[i][h],
                                  in_=xqT[rr(i), h * 512:(h + 1) * 512])
        for i in range(DC):
            nc.gpsimd.dma_start(out=wk_sb[i][:, 0:128], in_=wkT[rr(i), 0:128])
        for h in range(2):
            for i in range(DC):
                nc.gpsimd.dma_start(out=xk_sb[i][h],
                                    in_=xkT[rr(i), h * 512:(h + 1) * 512])
        for h in range(2):
            for i in range(DC):
                nc.gpsimd.dma_start(out=xv_sb[i][h],
                                    in_=xvT[rr(i), h * 512:(h + 1) * 512])
        for i in range(DC):
            nc.gpsimd.dma_start(out=wv_sb[i], in_=wvT[rr(i), :])
        for i in range(DC):
            nc.sync.dma_start(out=wq_sb[i][:, 128:HE], in_=wqT[rr(i), 128:HE])
            nc.sync.dma_start(out=wk_sb[i][:, 128:HE], in_=wkT[rr(i), 128:HE])

        # resident intermediates
        QT = [res.tile([128, T], mm_dt, tag=f"QT{i}", name=f"QT{i}")
              for i in range(PAIRS)]
        KT = [res.tile([128, T], mm_dt, tag=f"KT{i}", name=f"KT{i}")
              for i in range(PAIRS)]
        # V panel per k-chunk: col 0 = 1.0, cols 1..64 = V, col 65 = -1/lam
        VB = [res.tile([128, PAIRS, EWP], BF16, tag=f"VB{i}",
                       name=f"VB{i}") for i in range(KC)]
        ident = res.tile([128, 128], BF16, tag="ident", name="ident")
        make_identity(nc, ident)
        nlam_sb = res.tile([128, PAIRS], F32, tag="nlam", name="nlam_sb")
        nc.sync.dma_start(out=nlam_sb, in_=nlam)
        for i in range(KC):
            nc.vector.memset(VB[i][:, :, 0:1], 1.0)
            nc.vector.memset(VB[i][:, :, EW:EWP], 0.0)
            nc.vector.tensor_copy(VB[i][:, :, E + 1:E + 2], nlam_sb)

        # ---- V projection for one 128-key chunk ----
        def emit_v_chunk(tcn):
            ps = ppsum.tile([128, 512], F32, tag="ps", name="psv")
            h, t4 = tcn // 4, tcn % 4
            for dc in range(DC):
                mm(ps, xv_sb[dc][h][:, t4 * 128:(t4 + 1) * 128],
                   wv_sb[dc],
                   start=(dc == 0), stop=(dc == DC - 1))
            nc.vector.tensor_copy(VB[tcn][:, :, 1:E + 1],
                                  ps.rearrange("p (h e) -> p h e", e=E))

        # ---- QK projection ops for one pair (consumed a few per slot) ----
        def qk_proj_ops(p):
            ops = []
            csl = slice(p * 128, (p + 1) * 128)
            for (x_sb, w_sb, OUT) in ((xq_sb, wq_sb, QT), (xk_sb, wk_sb, KT)):
                for tq in range(T // 512):
                    ps = ppsum.tile([128, 512], F32, tag="ps", name="psqk")
                    for dc in range(DC):
                        ops.append((ps, w_sb[dc], csl, x_sb, tq, dc,
                                    OUT[p]))
            return ops

        def emit_proj(op):
            ps, w, csl, x, tq, dc, dst = op
            mm(ps, w[:, csl], x[dc][tq],
               start=(dc == 0), stop=(dc == DC - 1))
            if dc == DC - 1:
                nc.vector.tensor_copy(dst[:, tq * 512:(tq + 1) * 512], ps)

        # pair-0 QK projection upfront (overlaps input DMA)
        for op in qk_proj_ops(0):
            emit_proj(op)

        # ---- attention pipeline ----
        # One slot = (pair, qblock, kchunk): emit scores + exp, the AV
        # matmuls of the previous slot's e-tile, and filler projections.
        prev = None  # (e_tile, p, qb, kc)
        cur_o = {}   # (p, qb) -> (o_pos, o_neg)

        def emit_av_and_post(rec):
            e, pp_, qq_, kk_ = rec
            if kk_ == 0:
                o_pos = o_pool.tile([EWP, 512], F32, tag="o", name="o_pos")
                o_neg = o_pool.tile([EWP, 512], F32, tag="o", name="o_neg")
                cur_o[(pp_, qq_)] = (o_pos, o_neg)
            o_pos, o_neg = cur_o[(pp_, qq_)]
            first = (kk_ == 0)
            last = (kk_ == KC - 1)
            mm(o_pos, VB[kk_][:, pp_, :], e[:, 0, :], start=first, stop=last)
            mm(o_neg, VB[kk_][:, pp_, :], e[:, 1, :], start=first, stop=last)
            if last:
                emit_post(pp_, qq_, o_pos, o_neg)
                del cur_o[(pp_, qq_)]

        def emit_post(pp_, qq_, o_pos, o_neg):
            # o_pos rows: [den_pos, O_pos(64), junk]
            # o_neg rows: [junk,    O_neg(64), -den_neg/lam]
            osb = post_pool.tile([EWP, 2, 512], BF16, tag="osb", name="osb")
            nc.vector.tensor_copy(osb[:, 0, :], o_pos)
            nc.vector.tensor_copy(osb[:, 1, :], o_neg)
            tr = o_pool.tile([128, 2, QT4, EWP], BF16, tag="o", name="tr")
            for qt in range(QT4):
                tsl = slice(qt * 128, (qt + 1) * 128)
                nc.tensor.transpose(tr[:, 0, qt, :], osb[:, 0, tsl],
                                    ident[0:EWP, 0:EWP])
                nc.tensor.transpose(tr[:, 1, qt, :], osb[:, 1, tsl],
                                    ident[0:EWP, 0:EWP])
            rp = post_pool.tile([128, QT4], F32, tag="rp", name="rp")
            rn = post_pool.tile([128, QT4], F32, tag="rn", name="rn")
            nc.vector.reciprocal(rp, tr[:, 0, :, 0:1])
            nc.vector.reciprocal(rn, tr[:, 1, :, E + 1:E + 2])
            ot = post_pool.tile([128, QT4, E], F32, tag="ot", name="ot")
            for qt in range(QT4):
                nc.vector.tensor_scalar_mul(ot[:, qt, :],
                                            tr[:, 0, qt, 1:E + 1],
                                            rp[:, qt:qt + 1])
                nc.vector.scalar_tensor_tensor(
                    ot[:, qt, :], tr[:, 1, qt, 1:E + 1], rn[:, qt:qt + 1],
                    ot[:, qt, :],
                    op0=mybir.AluOpType.mult,
                    op1=mybir.AluOpType.add)
            nc.sync.dma_start(
                out=out[qq_ * 512:(qq_ + 1) * 512, pp_ * E:(pp_ + 1) * E]
                .rearrange("(qt r) e -> r qt e", qt=QT4),
                in_=ot)

        for p in range(PAIRS):
            pending = qk_proj_ops(p + 1) if p + 1 < PAIRS else []
            pi = 0
            for qb in range(QB):
                qsl = slice(qb * 512, (qb + 1) * 512)
                for kc in range(KC):
                    ksl = slice(kc * 128, (kc + 1) * 128)
                    s = s_pool.tile([128, 2, 512], F32, tag="s", name="s")
                    mm(s[:, 0, :], KT[p][0:64, ksl], QT[p][0:64, qsl],
                       start=True, stop=True, tile_position=(0, 0))
                    mm(s[:, 1, :], KT[p][64:128, ksl], QT[p][64:128, qsl],
                       start=True, stop=True, tile_position=(64, 0))
                    e = pexp_pool.tile([128, 2, 512], BF16, tag="e",
                                       name="e")
                    nc.scalar.activation(e, s, EXP)
                    # fill the exp-wait bubble with projection work
                    if p == 0 and qb == 0:
                        emit_v_chunk(kc)
                    else:
                        nproj = 4 if (p == 0 and qb == 1) else 2
                        for _ in range(nproj):
                            if pi < len(pending):
                                emit_proj(pending[pi])
                                pi += 1
                    if prev is not None:
                        emit_av_and_post(prev)
                    prev = (e, p, qb, kc)
            # drain leftover projection work before the next pair needs it
            while pi < len(pending):
                emit_proj(pending[pi])
                pi += 1
        # flush the last e-tile
        emit_av_and_post(prev)

    nc.compile()
    return nc


def make_in_maps(q_input, k_input, v_input, Wq, Wk, Wv, L):
    scale = np.float32(E ** -0.25)
    lam = (0.2 + np.exp(np.float32(L[0] @ L[1]))
           - np.exp(np.float32(L[2] @ L[3])))
    ninvlam = np.full((128, PAIRS), -1.0 / lam, np.float32)
    in_maps = []
    for c in range(N_CORES):
        b, hb = c // 2, c % 2
        in_maps.append({
            "xqT": np.ascontiguousarray(q_input[b].T).astype(np.float16),
            "xkT": np.ascontiguousarray(k_input[b].T).astype(np.float16),
            "xvT": np.ascontiguousarray(v_input[b].T).astype(np.float16),
            "wqT": (np.ascontiguousarray(Wq[1024 * hb:1024 * (hb + 1), :].T)
                    * scale).astype(np.float16),
            "wkT": (np.ascontiguousarray(Wk[1024 * hb:1024 * (hb + 1), :].T)
                    * scale).astype(np.float16),
            "wvT": np.ascontiguousarray(
                Wv[512 * hb:512 * (hb + 1), :].T).astype(np.float16),
            "nlam": ninvlam,
        })
    return in_maps


_NC_CACHE = {}


def get_nc(mm_dt=F16):
    key = str(mm_dt)
    if key not in _NC_CACHE:
        _NC_CACHE[key] = build_bass(mm_dt)
    return _NC_CACHE[key]


def kernel(q_input, k_input, v_input, Wq, Wk, Wv, L, _trace=False):
    q_input = np.asarray(q_input, np.float32)
    k_input = np.asarray(k_input, np.float32)
    v_input = np.asarray(v_input, np.float32)
    Wq = np.asarray(Wq, np.float32)
    Wk = np.asarray(Wk, np.float32)
    Wv = np.asarray(Wv, np.float32)
    L = np.asarray(L, np.float32)

    nc = get_nc()
    in_maps = make_in_maps(q_input, k_input, v_input, Wq, Wk, Wv, L)
    res = run_bass_kernel_spmd(nc, in_maps, list(range(N_CORES)), trace=_trace)

    full = np.empty((B, T, H * E), np.float32)
    for c in range(N_CORES):
        b, hb = c // 2, c % 2
        full[b, :, 512 * hb:512 * (hb + 1)] = res.results[c]["out"]
    if _trace:
        return full, res
    return full
